# revision 14
# baseline (speedup 1.0000x reference)
"""4-layer GCN (DGL GraphConv norm='both') on 8 Trainium2 NeuronCores.

Strategy (graph/data parallel, per sharding hint):
- Nodes partitioned into 8 contiguous shards of 12500 (padded to 12544 = 98*128).
  Core c owns node shard c and all edges whose dst lies in shard c.
- Per layer: each core computes the dense transform u = (x*ns) @ W for its
  shard (PE matmul, x kept transposed [feat, node] in SBUF; ns = D_src^-1/2
  applied as a per-partition activation scale on the node-major PSUM tile),
  writes node-major bf16 rows to DRAM. The AllGather is split into two
  half-shard collectives so the first overlaps the second half of the dense
  transform and the second overlaps early aggregation gathers.
- Aggregation: edges sorted by (dst tile, src window); rows u[src] fetched with
  gpsimd.dma_gather (<=1024 rows/instruction, int16 window-local indices,
  4 SWDGE queues round-robined); segment-sum by dst via pure 0/1 one-hot
  selection matrices (iota == dst_local; padded slots use dst_local=-1) built
  on the vector engine, then PE matmuls accumulating in PSUM [feat, dst].
- Epilogue per dst tile: acc *= nd (D_dst^-1/2, partition-replicated resident
  tile) on the vector engine, then relu(acc + bias) on the scalar engine
  writes the next layer's transposed input directly into SBUF. Layer 4 adds
  bias only and DMAs the [64, nodes] output (host transposes back).
"""
import os
if os.environ.get("JAX_PLATFORMS") == "cpu":
    # bass runs through the axon PJRT platform; a cpu-only pin would hide it
    os.environ["JAX_PLATFORMS"] = ""
import numpy as np
import ml_dtypes

import concourse.mybir as mybir
import concourse.tile as tile
from concourse.bacc import Bacc
from concourse.bass_utils import run_bass_kernel_spmd

P = 128
NCORES = 8
NNODES = 100000
SHN = 12500          # real nodes per core
NT = 98              # dst tiles per core
SH = NT * P          # padded nodes per core = 12544
HSH = SH // 2        # half shard = 6272 (49 tiles)
HROWS = NCORES * SH  # padded global rows = 100352
HHALF = HROWS // 2   # rows per half tensor = 50176
F = 128
FO = 64
NW = 4
WIN = HROWS // NW    # 25088 rows per int16 window
GB = 8               # blocks (of 128 rows) per dma_gather = 1024 idx
NQ = 4               # SWDGE queues

bf16 = mybir.dt.bfloat16
f32 = mybir.dt.float32
i16 = mybir.dt.int16

bf16_np = ml_dtypes.bfloat16


def _make_plan(cnt_max):
    """cnt_max: [NT, NW] max-over-cores edge counts per (dst tile, src window).

    Densely packed window-major stream: for w: cells (t, w) of exactly
    cnt_max[t, w] slots laid back to back (no rounding to 128), the window's
    stream padded to a 128 multiple at its end only. Blocks of 128 slots may
    straddle cell boundaries; each (block x cell) intersection is a "run" and
    gets one matmul whose one-hot rhs masks non-member partitions via
    dst_local=-1. Each cell is one PSUM accumulation segment (start on its
    first run, stop on its last). Gathers chunk each window's blocks into
    <=GB consecutive blocks; runs are numbered globally in emission order."""
    cnt_max = cnt_max.copy()
    for t in range(NT):
        if cnt_max[t].sum() == 0:
            cnt_max[t, 0] = 1  # force an (all-masked) run so acc[t] is written
    cell_start = np.zeros((NT, NW), np.int64)  # global slot of cell start
    nblk = 0
    runs = []      # (tile, seg_first, seg_last) in global emission order
    gathers = []   # (w, first_block, gb, run_lo, run_hi)
    blk_runs = []  # per block: list of run indices
    for w in range(NW):
        pos = nblk * P
        cells = []
        for t in range(NT):
            c = int(cnt_max[t, w])
            if c == 0:
                continue
            cell_start[t, w] = pos
            cells.append((t, pos, pos + c))
            pos += c
        wblk0 = nblk
        nblk = -(-pos // P)
        this_blocks = [[] for _ in range(wblk0, nblk)]
        for (t, s0, s1) in cells:
            b0, b1 = s0 // P, (s1 - 1) // P
            for b in range(b0, b1 + 1):
                c0 = max(s0, b * P) - b * P
                c1 = min(s1, (b + 1) * P) - b * P
                ri = len(runs)
                runs.append((t, b == b0, b == b1))
                this_blocks[b - wblk0].append((ri, c0, c1))
        blk_runs.extend(this_blocks)
        cur = wblk0
        while cur < nblk:
            gb = min(GB, nblk - cur)
            rlo = blk_runs[cur][0][0]
            rhi = blk_runs[cur + gb - 1][-1][0] + 1
            gathers.append((w, cur, gb, rlo, rhi))
            cur += gb
    nrg_max = max(g[4] - g[3] for g in gathers)
    return dict(cell_start=cell_start, runs=runs, gathers=gathers,
                blk_runs=blk_runs, nblk=nblk, nrun=len(runs),
                nrg_max=nrg_max)


def _build_kernel(plan):
    nblk = plan["nblk"]
    nrun = plan["nrun"]
    nrg_max = plan["nrg_max"]
    nc = Bacc("TRN2", num_devices=NCORES, num_swdge_queues=NQ)

    xT0_in = nc.dram_tensor("xT0", [P, SH], bf16, kind="ExternalInput")
    idx_in = nc.dram_tensor("idxs", [P, nblk * 8], i16, kind="ExternalInput")
    meta_in = nc.dram_tensor("meta", [P, nrg_max * P + nrun], bf16,
                             kind="ExternalInput")
    wts_in = nc.dram_tensor("wts", [P, 4 * P], bf16, kind="ExternalInput")
    bias_in = nc.dram_tensor("biases", [P, 4], f32, kind="ExternalInput")
    ns_in = nc.dram_tensor("nsn", [P, NT], f32, kind="ExternalInput")
    nd_in = nc.dram_tensor("ndr", [P, SH], bf16, kind="ExternalInput")
    out_t = nc.dram_tensor("outT", [FO, SH], f32, kind="ExternalOutput")

    h_sh = [nc.dram_tensor(f"h_sh{l}", [SH, F], bf16) for l in range(4)]
    h_fa = [nc.dram_tensor(f"h_fa{l}", [HHALF, F], bf16, addr_space="Shared")
            for l in range(4)]
    h_fb = [nc.dram_tensor(f"h_fb{l}", [HHALF, F], bf16, addr_space="Shared")
            for l in range(4)]

    with tile.TileContext(nc) as tc:
        with (
            tc.tile_pool(name="res", bufs=1) as res,
            tc.tile_pool(name="hp", bufs=4) as hp,
            tc.tile_pool(name="gp", bufs=6) as gp,
            tc.tile_pool(name="sp", bufs=6) as sp,
            tc.tile_pool(name="op", bufs=3) as op,
            tc.tile_pool(name="dpp", bufs=2, space="PSUM") as dpp,
            tc.tile_pool(name="app", bufs=6, space="PSUM") as app,
        ):
            x_a = res.tile([P, SH], bf16)
            x_b = res.tile([P, SH], bf16)
            acc = res.tile([P, SH], bf16)
            idx_t = res.tile([P, nblk * 8], i16)
            meta_t = res.tile([P, nrg_max * P + nrun], bf16)
            wts_t = res.tile([P, 4 * P], bf16)
            bias_t = res.tile([P, 4], f32)
            ns_t = res.tile([P, NT], f32)
            nd_t = res.tile([P, SH], bf16)
            nc.sync.dma_start(out=x_a[:], in_=xT0_in[:])
            nc.sync.dma_start(out=idx_t[:], in_=idx_in[:])
            nc.sync.dma_start(out=meta_t[:], in_=meta_in[:])
            nc.sync.dma_start(out=wts_t[:], in_=wts_in[:])
            nc.sync.dma_start(out=bias_t[:], in_=bias_in[:])
            nc.sync.dma_start(out=ns_t[:], in_=ns_in[:])
            nc.sync.dma_start(out=nd_t[:], in_=nd_in[:])
            iota = meta_t[:, 0:nrg_max * P]
            dloc0 = nrg_max * P

            for l in range(4):
                x_in = x_a if l % 2 == 0 else x_b
                x_out = x_b if l % 2 == 0 else x_a

                # ---- dense: u = (x * ns) @ W_l, node-major bf16 rows to DRAM
                # (ns folded in as per-partition scale on the node-major tile);
                # AllGather split in half-shard chunks to overlap with compute.
                for half in range(2):
                    for t in range(half * (NT // 2), NT if half else NT // 2):
                        dps = dpp.tile([P, P], f32, space="PSUM", tag="dps")
                        nc.tensor.matmul(
                            out=dps[:],
                            lhsT=x_in[:, t * P:(t + 1) * P],
                            rhs=wts_t[:, l * P:(l + 1) * P],
                            start=True, stop=True)
                        hb = hp.tile([P, P], bf16, tag="hb")
                        nc.scalar.activation(
                            out=hb[:], in_=dps[:],
                            func=mybir.ActivationFunctionType.Copy,
                            bias=0.0, scale=ns_t[:, t:t + 1])
                        nc.sync.dma_start(
                            out=h_sh[l][t * P:(t + 1) * P, :], in_=hb[:])
                    h_half = h_fa[l] if half == 0 else h_fb[l]
                    nc.gpsimd.collective_compute(
                        "AllGather", mybir.AluOpType.bypass,
                        replica_groups=[list(range(NCORES))],
                        ins=[h_sh[l][half * HSH:(half + 1) * HSH, :].opt()],
                        outs=[h_half[:].opt()])

                # ---- aggregate: packed window-major stream; one matmul per
                # (block x cell) run, one-hot rhs masks non-member partitions
                seen = set()
                open_seg = {}
                for gi, (w, fb, gb, rlo, rhi) in enumerate(plan["gathers"]):
                    src_half = h_fa[l] if w < 2 else h_fb[l]
                    woff = (w % 2) * WIN
                    nrg = rhi - rlo
                    g_t = gp.tile([P, GB, F], bf16, tag="g")
                    nc.gpsimd.dma_gather(
                        out_ap=g_t[:, 0:gb, :],
                        in_ap=src_half[woff:woff + WIN, :],
                        idxs_ap=idx_t[:, fb * 8:(fb + gb) * 8],
                        num_idxs=gb * P, num_idxs_reg=gb * P,
                        elem_size=F, queue_num=gi % NQ)
                    s_t = sp.tile([P, nrg_max * P], bf16, tag="s")
                    s3 = s_t[:, 0:nrg * P].rearrange("p (r q) -> p r q", r=nrg)
                    nc.vector.tensor_tensor(
                        out=s3,
                        in0=iota[:, 0:nrg * P].rearrange("p (r q) -> p r q", r=nrg),
                        in1=meta_t[:, dloc0 + rlo:dloc0 + rhi]
                            .to_broadcast([P, nrg, P]),
                        op=mybir.AluOpType.is_equal)
                    for j in range(gb):
                        for (ri, c0, c1) in plan["blk_runs"][fb + j]:
                            t, sf, sl_ = plan["runs"][ri]
                            if sf:
                                seg = app.tile([P, P], f32, space="PSUM",
                                               tag="segps")
                                open_seg[t] = seg
                            seg = open_seg[t]
                            nc.tensor.matmul(
                                out=seg[:],
                                lhsT=g_t[:, j, :],
                                rhs=s_t[:, (ri - rlo) * P:(ri - rlo + 1) * P],
                                start=sf, stop=sl_)
                            if sl_:
                                del open_seg[t]
                                sl = acc[:, t * P:(t + 1) * P]
                                if t in seen:
                                    nc.vector.tensor_tensor(
                                        out=sl, in0=sl, in1=seg[:],
                                        op=mybir.AluOpType.add)
                                else:
                                    nc.vector.tensor_copy(out=sl, in_=seg[:])
                                    seen.add(t)

                for t in range(NT):
                    sl = acc[:, t * P:(t + 1) * P]
                    nd_sl = nd_t[:, t * P:(t + 1) * P]
                    if l < 3:
                        nc.vector.tensor_tensor(
                            out=sl, in0=sl, in1=nd_sl,
                            op=mybir.AluOpType.mult)
                        nc.scalar.activation(
                            out=x_out[:, t * P:(t + 1) * P],
                            in_=sl,
                            func=mybir.ActivationFunctionType.Relu,
                            bias=bias_t[:, l:l + 1], scale=1.0)
                    else:
                        o_t = op.tile([FO, P], f32, tag="o")
                        nc.vector.tensor_tensor(
                            out=o_t[:],
                            in0=sl[0:FO, :], in1=nd_sl[0:FO, :],
                            op=mybir.AluOpType.mult)
                        nc.vector.tensor_tensor(
                            out=o_t[:],
                            in0=o_t[:],
                            in1=bias_t[0:FO, 3:4].to_broadcast([FO, P]),
                            op=mybir.AluOpType.add)
                        nc.sync.dma_start(
                            out=out_t[:, t * P:(t + 1) * P], in_=o_t[:])
    nc.finalize()
    return nc


def _wrap_idx16(stream):
    """element i -> [i % 16, i // 16], replicated to 128 partitions."""
    a = stream.reshape(-1, 16).T  # [16, n/16]
    return np.tile(a, (8, 1))


def _run_timed(nc, in_maps, iters):
    """Run the SPMD program via PJRT like run_bass_kernel_spmd's axon path,
    but keep the compiled executable + device-resident inputs so repeated
    executions measure on-device time (compile and H2D excluded)."""
    import time
    import jax
    from jax.sharding import Mesh, PartitionSpec, NamedSharding
    from jax.experimental.shard_map import shard_map
    from concourse import bass2jax
    import concourse.mybir as _mb

    bass2jax.install_neuronx_cc_hook()
    n_cores = len(in_maps)
    in_names, out_names, out_avals, zero_outs = [], [], [], []
    for alloc in nc.m.functions[0].allocations:
        if not isinstance(alloc, _mb.MemoryLocationSet):
            continue
        name = alloc.memorylocations[0].name
        if alloc.kind == "ExternalInput":
            if nc.partition_id_tensor is None or name != nc.partition_id_tensor.name:
                in_names.append(name)
        elif alloc.kind == "ExternalOutput":
            shape = tuple(alloc.tensor_shape)
            dtype = _mb.dt.np(alloc.dtype)
            out_names.append(name)
            out_avals.append(jax.core.ShapedArray(shape, dtype))
            zero_outs.append(np.zeros(shape, dtype))
    n_params = len(in_names)
    n_outs = len(out_avals)
    all_in_names = in_names + out_names
    if nc.partition_id_tensor is not None:
        all_in_names.append(nc.partition_id_tensor.name)

    def _body(*args):
        operands = list(args)
        if nc.partition_id_tensor is not None:
            operands.append(bass2jax.partition_id_tensor())
        outs = bass2jax._bass_exec_p.bind(
            *operands,
            out_avals=tuple(out_avals),
            in_names=tuple(all_in_names),
            out_names=tuple(out_names),
            lowering_input_output_aliases=(),
            sim_require_finite=True,
            sim_require_nnan=True,
            nc=nc,
        )
        return tuple(outs)

    devices = jax.devices()[:n_cores]
    mesh = Mesh(np.asarray(devices), ("core",))
    donate = tuple(range(n_params, n_params + n_outs))
    sharded = jax.jit(
        shard_map(_body, mesh=mesh,
                  in_specs=(PartitionSpec("core"),) * (n_params + n_outs),
                  out_specs=(PartitionSpec("core"),) * n_outs,
                  check_rep=False),
        donate_argnums=donate, keep_unused=True)
    spec = NamedSharding(mesh, PartitionSpec("core"))
    dev_in = [jax.device_put(
        np.concatenate([np.asarray(m[name]) for m in in_maps], axis=0), spec)
        for name in in_names]
    cur = tuple(jax.device_put(
        np.zeros((n_cores * z.shape[0], *z.shape[1:]), z.dtype), spec)
        for z in zero_outs)
    cur = sharded(*dev_in, *cur)
    jax.block_until_ready(cur)
    outs_np = [np.asarray(o) for o in cur]
    timing = {}
    if iters > 0:
        # independent donated output buffers per iteration -> executions
        # pipeline on device; dispatch RPC overhead overlaps.
        def fresh_outs():
            return tuple(jax.device_put(
                np.zeros((n_cores * z.shape[0], *z.shape[1:]), z.dtype), spec)
                for z in zero_outs)
        bufs = [fresh_outs() for _ in range(iters)]
        warm = [fresh_outs() for _ in range(2)]
        jax.block_until_ready(bufs)
        rs = [sharded(*dev_in, *b) for b in warm]
        jax.block_until_ready(rs)
        t0 = time.perf_counter()
        rs = [sharded(*dev_in, *b) for b in bufs]
        jax.block_until_ready(rs)
        t1 = time.perf_counter()
        timing["per_iter_ns"] = (t1 - t0) / iters * 1e9
    results = [
        {name: outs_np[i].reshape(n_cores, *out_avals[i].shape)[c]
         for i, name in enumerate(out_names)}
        for c in range(n_cores)
    ]
    return results, timing


def _prepare(in_feat, src, dst, W1, b1, W2, b2, W3, b3, W4, b4):
    src = np.asarray(src).astype(np.int64)
    dst = np.asarray(dst).astype(np.int64)
    in_feat = np.asarray(in_feat, dtype=np.float32)

    deg_out = np.bincount(src, minlength=NNODES).astype(np.float32)
    deg_in = np.bincount(dst, minlength=NNODES).astype(np.float32)
    ns = (1.0 / np.sqrt(np.maximum(deg_out, 1.0))).astype(np.float32)
    nd = (1.0 / np.sqrt(np.maximum(deg_in, 1.0))).astype(np.float32)

    c_e = dst // SHN
    loc = dst % SHN
    t_e = loc // P
    dl_e = (loc % P).astype(np.float32)
    # src row in the half-gathered layout: half h of core c sits at
    # rows [h*HHALF + c*HSH, ...) of the (h==0 ? A : B) tensor pair.
    sc = src // SHN
    sj = src % SHN
    sh_half = sj // HSH
    gr = sh_half * HHALF + sc * HSH + (sj - sh_half * HSH)
    w_e = gr // WIN
    li_e = (gr - w_e * WIN).astype(np.int64)

    key = (c_e * NT + t_e) * NW + w_e
    cnt = np.bincount(key, minlength=NCORES * NT * NW).reshape(NCORES, NT, NW)
    plan = _make_plan(cnt.max(axis=0).astype(np.int64))
    nblk = plan["nblk"]
    nrun = plan["nrun"]
    nrg_max = plan["nrg_max"]
    cell_start = plan["cell_start"]

    iota_rep = np.tile(np.arange(P, dtype=np.float32), (P, nrg_max))
    wts = np.zeros((P, 4 * P), np.float32)
    for i, W in enumerate((W1, W2, W3, W4)):
        W = np.asarray(W, np.float32)
        wts[:, i * P:i * P + W.shape[1]] = W
    biases = np.zeros((P, 4), np.float32)
    for i, b in enumerate((b1, b2, b3, b4)):
        b = np.asarray(b, np.float32)
        biases[:b.shape[0], i] = b

    in_maps = []
    for c in range(NCORES):
        m = c_e == c
        et, ew = t_e[m], w_e[m]
        eli, edl = li_e[m], dl_e[m]
        order = np.lexsort((ew, et))
        et, ew = et[order], ew[order]
        eli, edl = eli[order], edl[order]
        bkey = et * NW + ew
        _, start_pos, bcnt = np.unique(bkey, return_index=True, return_counts=True)
        rank = np.arange(len(bkey)) - np.repeat(start_pos, bcnt)
        slot = cell_start[et, ew] + rank

        idx_stream = np.zeros(nblk * P, np.int16)
        idx_stream[slot] = eli.astype(np.int16)
        # unfilled slots: dst_local = -1 -> is_equal never fires -> zero column
        dloc_slot = np.full(nblk * P, -1.0, np.float32)
        dloc_slot[slot] = edl

        # per-run one-hot key columns: run ri covers block b cols [c0, c1);
        # partitions outside the run stay -1 (masked out of its matmul)
        dloc_dev = np.full((P, nrun), -1.0, np.float32)
        for b, rl in enumerate(plan["blk_runs"]):
            for (ri, c0, c1) in rl:
                dloc_dev[c0:c1, ri] = dloc_slot[b * P + c0:b * P + c1]
        meta = np.concatenate([iota_rep, dloc_dev], axis=1)

        # per-node norms for this shard (0 on padding rows)
        ns_sh = np.zeros(SH, np.float32)
        ns_sh[:SHN] = ns[c * SHN:(c + 1) * SHN]
        nd_sh = np.zeros(SH, np.float32)
        nd_sh[:SHN] = nd[c * SHN:(c + 1) * SHN]
        ns_dev = ns_sh.reshape(NT, P).T            # [P, NT] node-major scale
        nd_rep = np.tile(nd_sh[None, :], (P, 1))   # [P, SH] column multiplier

        sh = in_feat[c * SHN:(c + 1) * SHN]
        xT0 = np.zeros((P, SH), np.float32)
        xT0[:, :SHN] = sh.T

        in_maps.append({
            "xT0": xT0.astype(bf16_np),
            "idxs": _wrap_idx16(idx_stream),
            "meta": meta.astype(bf16_np),
            "wts": wts.astype(bf16_np),
            "biases": biases,
            "nsn": ns_dev,
            "ndr": nd_rep.astype(bf16_np),
        })

    return plan, in_maps


def kernel(in_feat, src, dst, W1, b1, W2, b2, W3, b3, W4, b4):
    plan, in_maps = _prepare(in_feat, src, dst, W1, b1, W2, b2, W3, b3, W4, b4)
    nc = _build_kernel(plan)
    iters = int(os.environ.get("KERNEL_TIME_ITERS", "0"))
    if iters > 0:
        results, timing = _run_timed(nc, in_maps, iters)
        if os.environ.get("KERNEL_RESULT_PATH"):
            import pickle
            with open(os.environ["KERNEL_RESULT_PATH"], "wb") as f:
                pickle.dump({"exec_time_ns": timing.get("per_iter_ns")}, f)
    else:
        res = run_bass_kernel_spmd(nc, in_maps, core_ids=list(range(NCORES)))
        results = res.results

    out = np.concatenate(
        [results[c]["outT"].T[:SHN] for c in range(NCORES)], axis=0)
    return np.ascontiguousarray(out, dtype=np.float32)


# revision 18
# speedup vs baseline: 1.1600x; 1.1600x over previous
"""4-layer GCN (DGL GraphConv norm='both') on 8 Trainium2 NeuronCores.

Strategy (graph/data parallel, per sharding hint):
- Nodes partitioned into 8 contiguous shards of 12500 (padded to 12544 = 98*128).
  Core c owns node shard c and all edges whose dst lies in shard c.
- Per layer: each core computes the dense transform u = (x*ns) @ W for its
  shard (PE matmul, x kept transposed [feat, node] in SBUF; ns = D_src^-1/2
  applied as a per-partition activation scale on the node-major PSUM tile),
  writes node-major bf16 rows to DRAM. The AllGather is split into two
  half-shard collectives so the first overlaps the second half of the dense
  transform and the second overlaps early aggregation gathers.
- Aggregation: edges sorted by (dst tile, src window); rows u[src] fetched with
  gpsimd.dma_gather (<=1024 rows/instruction, int16 window-local indices,
  4 SWDGE queues round-robined); segment-sum by dst via pure 0/1 one-hot
  selection matrices (iota == dst_local; padded slots use dst_local=-1) built
  on the vector engine, then PE matmuls accumulating in PSUM [feat, dst].
- Epilogue per dst tile: acc *= nd (D_dst^-1/2, partition-replicated resident
  tile) on the vector engine, then relu(acc + bias) on the scalar engine
  writes the next layer's transposed input directly into SBUF. Layer 4 adds
  bias only and DMAs the [64, nodes] output (host transposes back).
"""
import os
if os.environ.get("JAX_PLATFORMS") == "cpu":
    # bass runs through the axon PJRT platform; a cpu-only pin would hide it
    os.environ["JAX_PLATFORMS"] = ""
import numpy as np
import ml_dtypes

import concourse.mybir as mybir
import concourse.tile as tile
from concourse.bacc import Bacc
from concourse.bass_utils import run_bass_kernel_spmd

P = 128
NCORES = 8
NNODES = 100000
SHN = 12500          # real nodes per core
NT = 98              # dst tiles per core
SH = NT * P          # padded nodes per core = 12544
HSH = SH // 2        # half shard = 6272 (49 tiles)
HROWS = NCORES * SH  # padded global rows = 100352
HHALF = HROWS // 2   # rows per half tensor = 50176
F = 128
FO = 64
NW = 4
WIN = HROWS // NW    # 25088 rows per int16 window
GB = 8               # blocks (of 128 rows) per dma_gather = 1024 idx
NQ = 4               # SWDGE queues

bf16 = mybir.dt.bfloat16
f32 = mybir.dt.float32
i16 = mybir.dt.int16

bf16_np = ml_dtypes.bfloat16


def _make_plan(cnt_max):
    """cnt_max: [NT, NW] max-over-cores edge counts per (dst tile, src window).

    Densely packed window-major stream: for w: cells (t, w) of exactly
    cnt_max[t, w] slots laid back to back (no rounding to 128), the window's
    stream padded to a 128 multiple at its end only. Blocks of 128 slots may
    straddle cell boundaries; each (block x cell) intersection is a "run" and
    gets one matmul whose one-hot rhs masks non-member partitions via
    dst_local=-1. Each cell is one PSUM accumulation segment (start on its
    first run, stop on its last). Gathers chunk each window's blocks into
    <=GB consecutive blocks; runs are numbered globally in emission order."""
    cnt_max = cnt_max.copy()
    for t in range(NT):
        if cnt_max[t].sum() == 0:
            cnt_max[t, 0] = 1  # force an (all-masked) run so acc[t] is written
    cell_start = np.zeros((NT, NW), np.int64)  # global slot of cell start
    nblk = 0
    runs = []      # (tile, seg_first, seg_last) in global emission order
    gathers = []   # (w, first_block, gb, run_lo, run_hi)
    blk_runs = []  # per block: list of run indices
    for w in range(NW):
        pos = nblk * P
        cells = []
        for t in range(NT):
            c = int(cnt_max[t, w])
            if c == 0:
                continue
            cell_start[t, w] = pos
            cells.append((t, pos, pos + c))
            pos += c
        wblk0 = nblk
        nblk = -(-pos // P)
        this_blocks = [[] for _ in range(wblk0, nblk)]
        for (t, s0, s1) in cells:
            b0, b1 = s0 // P, (s1 - 1) // P
            for b in range(b0, b1 + 1):
                c0 = max(s0, b * P) - b * P
                c1 = min(s1, (b + 1) * P) - b * P
                ri = len(runs)
                runs.append((t, b == b0, b == b1))
                this_blocks[b - wblk0].append((ri, c0, c1))
        blk_runs.extend(this_blocks)
        cur = wblk0
        while cur < nblk:
            gb = min(GB, nblk - cur)
            rlo = blk_runs[cur][0][0]
            rhi = blk_runs[cur + gb - 1][-1][0] + 1
            gathers.append((w, cur, gb, rlo, rhi))
            cur += gb
    nrg_max = max(g[4] - g[3] for g in gathers)
    return dict(cell_start=cell_start, runs=runs, gathers=gathers,
                blk_runs=blk_runs, nblk=nblk, nrun=len(runs),
                nrg_max=nrg_max)


def _build_kernel(plan):
    nblk = plan["nblk"]
    nrun = plan["nrun"]
    nrg_max = plan["nrg_max"]
    nc = Bacc("TRN2", num_devices=NCORES, num_swdge_queues=NQ,
              dynamic_dma_scratch_size=32768)

    xT0_in = nc.dram_tensor("xT0", [P, SH], bf16, kind="ExternalInput")
    idx_in = nc.dram_tensor("idxs", [P, nblk * 8], i16, kind="ExternalInput")
    meta_in = nc.dram_tensor("meta", [P, nrg_max * P + nrun], bf16,
                             kind="ExternalInput")
    wts_in = nc.dram_tensor("wts", [P, 4 * P], bf16, kind="ExternalInput")
    bias_in = nc.dram_tensor("biases", [P, 4], f32, kind="ExternalInput")
    ns_in = nc.dram_tensor("nsn", [P, NT], f32, kind="ExternalInput")
    nd_in = nc.dram_tensor("ndr", [P, SH], bf16, kind="ExternalInput")
    out_t = nc.dram_tensor("outT", [FO, SH], f32, kind="ExternalOutput")

    h_sh = [nc.dram_tensor(f"h_sh{l}", [SH, F], bf16) for l in range(4)]
    h_fa = [nc.dram_tensor(f"h_fa{l}", [HHALF, F], bf16, addr_space="Shared")
            for l in range(4)]
    h_fb = [nc.dram_tensor(f"h_fb{l}", [HHALF, F], bf16, addr_space="Shared")
            for l in range(4)]

    with tile.TileContext(nc) as tc:
        with (
            tc.tile_pool(name="res", bufs=1) as res,
            tc.tile_pool(name="hp", bufs=4) as hp,
            tc.tile_pool(name="gp", bufs=6) as gp,
            tc.tile_pool(name="sp", bufs=6) as sp,
            tc.tile_pool(name="op", bufs=3) as op,
            tc.tile_pool(name="dpp", bufs=2, space="PSUM") as dpp,
            tc.tile_pool(name="app", bufs=6, space="PSUM") as app,
        ):
            x_a = res.tile([P, SH], bf16)
            x_b = res.tile([P, SH], bf16)
            acc = res.tile([P, SH], bf16)
            idx_t = res.tile([P, nblk * 8], i16)
            meta_t = res.tile([P, nrg_max * P + nrun], bf16)
            wts_t = res.tile([P, 4 * P], bf16)
            bias_t = res.tile([P, 4], f32)
            ns_t = res.tile([P, NT], f32)
            nd_t = res.tile([P, SH], bf16)
            nc.sync.dma_start(out=x_a[:], in_=xT0_in[:])
            nc.sync.dma_start(out=idx_t[:], in_=idx_in[:])
            nc.sync.dma_start(out=meta_t[:], in_=meta_in[:])
            nc.sync.dma_start(out=wts_t[:], in_=wts_in[:])
            nc.sync.dma_start(out=bias_t[:], in_=bias_in[:])
            nc.sync.dma_start(out=ns_t[:], in_=ns_in[:])
            nc.sync.dma_start(out=nd_t[:], in_=nd_in[:])
            iota = meta_t[:, 0:nrg_max * P]
            dloc0 = nrg_max * P

            for l in range(4):
                x_in = x_a if l % 2 == 0 else x_b
                x_out = x_b if l % 2 == 0 else x_a

                # ---- dense: u = (x * ns) @ W_l, node-major bf16 rows to DRAM
                # (ns folded in as per-partition scale on the node-major tile);
                # AllGather split in half-shard chunks to overlap with compute.
                for half in range(2):
                    for t in range(half * (NT // 2), NT if half else NT // 2):
                        dps = dpp.tile([P, P], f32, space="PSUM", tag="dps")
                        nc.tensor.matmul(
                            out=dps[:],
                            lhsT=x_in[:, t * P:(t + 1) * P],
                            rhs=wts_t[:, l * P:(l + 1) * P],
                            start=True, stop=True)
                        hb = hp.tile([P, P], bf16, tag="hb")
                        nc.scalar.activation(
                            out=hb[:], in_=dps[:],
                            func=mybir.ActivationFunctionType.Copy,
                            bias=0.0, scale=ns_t[:, t:t + 1])
                        nc.sync.dma_start(
                            out=h_sh[l][t * P:(t + 1) * P, :], in_=hb[:])
                    h_half = h_fa[l] if half == 0 else h_fb[l]
                    nc.gpsimd.collective_compute(
                        "AllGather", mybir.AluOpType.bypass,
                        replica_groups=[list(range(NCORES))],
                        ins=[h_sh[l][half * HSH:(half + 1) * HSH, :].opt()],
                        outs=[h_half[:].opt()])

                # ---- aggregate: packed window-major stream; one matmul per
                # (block x cell) run, one-hot rhs masks non-member partitions
                seen = set()
                open_seg = {}
                for gi, (w, fb, gb, rlo, rhi) in enumerate(plan["gathers"]):
                    src_half = h_fa[l] if w < 2 else h_fb[l]
                    woff = (w % 2) * WIN
                    nrg = rhi - rlo
                    g_t = gp.tile([P, GB, F], bf16, tag="g")
                    nc.gpsimd.dma_gather(
                        out_ap=g_t[:, 0:gb, :],
                        in_ap=src_half[woff:woff + WIN, :],
                        idxs_ap=idx_t[:, fb * 8:(fb + gb) * 8],
                        num_idxs=gb * P, num_idxs_reg=gb * P,
                        elem_size=F, queue_num=gi % NQ)
                    s_t = sp.tile([P, nrg_max * P], bf16, tag="s")
                    s3 = s_t[:, 0:nrg * P].rearrange("p (r q) -> p r q", r=nrg)
                    nc.vector.tensor_tensor(
                        out=s3,
                        in0=iota[:, 0:nrg * P].rearrange("p (r q) -> p r q", r=nrg),
                        in1=meta_t[:, dloc0 + rlo:dloc0 + rhi]
                            .to_broadcast([P, nrg, P]),
                        op=mybir.AluOpType.is_equal)
                    for j in range(gb):
                        for (ri, c0, c1) in plan["blk_runs"][fb + j]:
                            t, sf, sl_ = plan["runs"][ri]
                            if sf:
                                seg = app.tile([P, P], f32, space="PSUM",
                                               tag="segps")
                                open_seg[t] = seg
                            seg = open_seg[t]
                            nc.tensor.matmul(
                                out=seg[:],
                                lhsT=g_t[:, j, :],
                                rhs=s_t[:, (ri - rlo) * P:(ri - rlo + 1) * P],
                                start=sf, stop=sl_)
                            if sl_:
                                del open_seg[t]
                                sl = acc[:, t * P:(t + 1) * P]
                                if t in seen:
                                    nc.vector.tensor_tensor(
                                        out=sl, in0=sl, in1=seg[:],
                                        op=mybir.AluOpType.add)
                                else:
                                    nc.vector.tensor_copy(out=sl, in_=seg[:])
                                    seen.add(t)

                for t in range(NT):
                    sl = acc[:, t * P:(t + 1) * P]
                    nd_sl = nd_t[:, t * P:(t + 1) * P]
                    if l < 3:
                        nc.vector.tensor_tensor(
                            out=sl, in0=sl, in1=nd_sl,
                            op=mybir.AluOpType.mult)
                        nc.scalar.activation(
                            out=x_out[:, t * P:(t + 1) * P],
                            in_=sl,
                            func=mybir.ActivationFunctionType.Relu,
                            bias=bias_t[:, l:l + 1], scale=1.0)
                    else:
                        o_t = op.tile([FO, P], f32, tag="o")
                        nc.vector.tensor_tensor(
                            out=o_t[:],
                            in0=sl[0:FO, :], in1=nd_sl[0:FO, :],
                            op=mybir.AluOpType.mult)
                        nc.vector.tensor_tensor(
                            out=o_t[:],
                            in0=o_t[:],
                            in1=bias_t[0:FO, 3:4].to_broadcast([FO, P]),
                            op=mybir.AluOpType.add)
                        nc.sync.dma_start(
                            out=out_t[:, t * P:(t + 1) * P], in_=o_t[:])
    nc.finalize()
    return nc


def _wrap_idx16(stream):
    """element i -> [i % 16, i // 16], replicated to 128 partitions."""
    a = stream.reshape(-1, 16).T  # [16, n/16]
    return np.tile(a, (8, 1))


def _run_timed(nc, in_maps, iters):
    """Run the SPMD program via PJRT like run_bass_kernel_spmd's axon path,
    but keep the compiled executable + device-resident inputs so repeated
    executions measure on-device time (compile and H2D excluded)."""
    import time
    import jax
    from jax.sharding import Mesh, PartitionSpec, NamedSharding
    from jax.experimental.shard_map import shard_map
    from concourse import bass2jax
    import concourse.mybir as _mb

    bass2jax.install_neuronx_cc_hook()
    n_cores = len(in_maps)
    in_names, out_names, out_avals, zero_outs = [], [], [], []
    for alloc in nc.m.functions[0].allocations:
        if not isinstance(alloc, _mb.MemoryLocationSet):
            continue
        name = alloc.memorylocations[0].name
        if alloc.kind == "ExternalInput":
            if nc.partition_id_tensor is None or name != nc.partition_id_tensor.name:
                in_names.append(name)
        elif alloc.kind == "ExternalOutput":
            shape = tuple(alloc.tensor_shape)
            dtype = _mb.dt.np(alloc.dtype)
            out_names.append(name)
            out_avals.append(jax.core.ShapedArray(shape, dtype))
            zero_outs.append(np.zeros(shape, dtype))
    n_params = len(in_names)
    n_outs = len(out_avals)
    all_in_names = in_names + out_names
    if nc.partition_id_tensor is not None:
        all_in_names.append(nc.partition_id_tensor.name)

    def _body(*args):
        operands = list(args)
        if nc.partition_id_tensor is not None:
            operands.append(bass2jax.partition_id_tensor())
        outs = bass2jax._bass_exec_p.bind(
            *operands,
            out_avals=tuple(out_avals),
            in_names=tuple(all_in_names),
            out_names=tuple(out_names),
            lowering_input_output_aliases=(),
            sim_require_finite=True,
            sim_require_nnan=True,
            nc=nc,
        )
        return tuple(outs)

    devices = jax.devices()[:n_cores]
    mesh = Mesh(np.asarray(devices), ("core",))
    donate = tuple(range(n_params, n_params + n_outs))
    sharded = jax.jit(
        shard_map(_body, mesh=mesh,
                  in_specs=(PartitionSpec("core"),) * (n_params + n_outs),
                  out_specs=(PartitionSpec("core"),) * n_outs,
                  check_rep=False),
        donate_argnums=donate, keep_unused=True)
    spec = NamedSharding(mesh, PartitionSpec("core"))
    dev_in = [jax.device_put(
        np.concatenate([np.asarray(m[name]) for m in in_maps], axis=0), spec)
        for name in in_names]
    cur = tuple(jax.device_put(
        np.zeros((n_cores * z.shape[0], *z.shape[1:]), z.dtype), spec)
        for z in zero_outs)
    cur = sharded(*dev_in, *cur)
    jax.block_until_ready(cur)
    outs_np = [np.asarray(o) for o in cur]
    timing = {}
    if iters > 0:
        # independent donated output buffers per iteration -> executions
        # pipeline on device; dispatch RPC overhead overlaps.
        def fresh_outs():
            return tuple(jax.device_put(
                np.zeros((n_cores * z.shape[0], *z.shape[1:]), z.dtype), spec)
                for z in zero_outs)
        bufs = [fresh_outs() for _ in range(iters)]
        warm = [fresh_outs() for _ in range(2)]
        jax.block_until_ready(bufs)
        rs = [sharded(*dev_in, *b) for b in warm]
        jax.block_until_ready(rs)
        t0 = time.perf_counter()
        rs = [sharded(*dev_in, *b) for b in bufs]
        jax.block_until_ready(rs)
        t1 = time.perf_counter()
        timing["per_iter_ns"] = (t1 - t0) / iters * 1e9
    results = [
        {name: outs_np[i].reshape(n_cores, *out_avals[i].shape)[c]
         for i, name in enumerate(out_names)}
        for c in range(n_cores)
    ]
    return results, timing


def _prepare(in_feat, src, dst, W1, b1, W2, b2, W3, b3, W4, b4):
    src = np.asarray(src).astype(np.int64)
    dst = np.asarray(dst).astype(np.int64)
    in_feat = np.asarray(in_feat, dtype=np.float32)

    deg_out = np.bincount(src, minlength=NNODES).astype(np.float32)
    deg_in = np.bincount(dst, minlength=NNODES).astype(np.float32)
    ns = (1.0 / np.sqrt(np.maximum(deg_out, 1.0))).astype(np.float32)
    nd = (1.0 / np.sqrt(np.maximum(deg_in, 1.0))).astype(np.float32)

    c_e = dst // SHN
    loc = dst % SHN
    t_e = loc // P
    dl_e = (loc % P).astype(np.float32)
    # src row in the half-gathered layout: half h of core c sits at
    # rows [h*HHALF + c*HSH, ...) of the (h==0 ? A : B) tensor pair.
    sc = src // SHN
    sj = src % SHN
    sh_half = sj // HSH
    gr = sh_half * HHALF + sc * HSH + (sj - sh_half * HSH)
    w_e = gr // WIN
    li_e = (gr - w_e * WIN).astype(np.int64)

    key = (c_e * NT + t_e) * NW + w_e
    cnt = np.bincount(key, minlength=NCORES * NT * NW).reshape(NCORES, NT, NW)
    plan = _make_plan(cnt.max(axis=0).astype(np.int64))
    nblk = plan["nblk"]
    nrun = plan["nrun"]
    nrg_max = plan["nrg_max"]
    cell_start = plan["cell_start"]

    iota_rep = np.tile(np.arange(P, dtype=np.float32), (P, nrg_max))
    wts = np.zeros((P, 4 * P), np.float32)
    for i, W in enumerate((W1, W2, W3, W4)):
        W = np.asarray(W, np.float32)
        wts[:, i * P:i * P + W.shape[1]] = W
    biases = np.zeros((P, 4), np.float32)
    for i, b in enumerate((b1, b2, b3, b4)):
        b = np.asarray(b, np.float32)
        biases[:b.shape[0], i] = b

    in_maps = []
    for c in range(NCORES):
        m = c_e == c
        et, ew = t_e[m], w_e[m]
        eli, edl = li_e[m], dl_e[m]
        order = np.lexsort((ew, et))
        et, ew = et[order], ew[order]
        eli, edl = eli[order], edl[order]
        bkey = et * NW + ew
        _, start_pos, bcnt = np.unique(bkey, return_index=True, return_counts=True)
        rank = np.arange(len(bkey)) - np.repeat(start_pos, bcnt)
        slot = cell_start[et, ew] + rank

        idx_stream = np.zeros(nblk * P, np.int16)
        idx_stream[slot] = eli.astype(np.int16)
        # unfilled slots: dst_local = -1 -> is_equal never fires -> zero column
        dloc_slot = np.full(nblk * P, -1.0, np.float32)
        dloc_slot[slot] = edl

        # per-run one-hot key columns: run ri covers block b cols [c0, c1);
        # partitions outside the run stay -1 (masked out of its matmul)
        dloc_dev = np.full((P, nrun), -1.0, np.float32)
        for b, rl in enumerate(plan["blk_runs"]):
            for (ri, c0, c1) in rl:
                dloc_dev[c0:c1, ri] = dloc_slot[b * P + c0:b * P + c1]
        meta = np.concatenate([iota_rep, dloc_dev], axis=1)

        # per-node norms for this shard (0 on padding rows)
        ns_sh = np.zeros(SH, np.float32)
        ns_sh[:SHN] = ns[c * SHN:(c + 1) * SHN]
        nd_sh = np.zeros(SH, np.float32)
        nd_sh[:SHN] = nd[c * SHN:(c + 1) * SHN]
        ns_dev = ns_sh.reshape(NT, P).T            # [P, NT] node-major scale
        nd_rep = np.tile(nd_sh[None, :], (P, 1))   # [P, SH] column multiplier

        sh = in_feat[c * SHN:(c + 1) * SHN]
        xT0 = np.zeros((P, SH), np.float32)
        xT0[:, :SHN] = sh.T

        in_maps.append({
            "xT0": xT0.astype(bf16_np),
            "idxs": _wrap_idx16(idx_stream),
            "meta": meta.astype(bf16_np),
            "wts": wts.astype(bf16_np),
            "biases": biases,
            "nsn": ns_dev,
            "ndr": nd_rep.astype(bf16_np),
        })

    return plan, in_maps


def kernel(in_feat, src, dst, W1, b1, W2, b2, W3, b3, W4, b4):
    plan, in_maps = _prepare(in_feat, src, dst, W1, b1, W2, b2, W3, b3, W4, b4)
    nc = _build_kernel(plan)
    iters = int(os.environ.get("KERNEL_TIME_ITERS", "0"))
    if iters > 0:
        results, timing = _run_timed(nc, in_maps, iters)
        if os.environ.get("KERNEL_RESULT_PATH"):
            import pickle
            with open(os.environ["KERNEL_RESULT_PATH"], "wb") as f:
                pickle.dump({"exec_time_ns": timing.get("per_iter_ns")}, f)
    else:
        res = run_bass_kernel_spmd(nc, in_maps, core_ids=list(range(NCORES)))
        results = res.results

    out = np.concatenate(
        [results[c]["outT"].T[:SHN] for c in range(NCORES)], axis=0)
    return np.ascontiguousarray(out, dtype=np.float32)
